# revision 6
# baseline (speedup 1.0000x reference)
"""GCNConv forward on 8 Trainium2 NeuronCores.

Math: out = dinv * (A_hat^T @ (dinv * x)) @ W + b, where A_hat = A + I and
dinv = 1/sqrt(1 + in-degree).  Aggregation commutes with the linear layer,
so we aggregate in the 128-dim input space and apply W afterwards.

Sharding: targets (output nodes) are partitioned 1250/core (no collectives).
The edge scatter is computed as a dense blocked GEMM: the host packs the
(padded) multiplicity matrix A[s, t] as fp8e4m3 (counts are small integers,
exact in fp8), blocked [p=128, sb=79, t=1250] per core.  Each core:
  xs = x * dinv_s                      (DVE, bf16)
  agg_T[f, t] += xs_sb^T @ A[sb]       (PE, 79 accumulating matmuls x 3 t-chunks)
  aggS = agg_T * dinv_t                (DVE)
  y[t, o] = aggS_tb^T @ W + b          (PE fp32 + DVE)
"""

import functools
import os

import numpy as np
import ml_dtypes

import concourse.bacc as bacc
import concourse.bass as bass
import concourse.mybir as mybir
import concourse.tile as tile
from concourse.bass_utils import run_bass_kernel_spmd

N = 10000
F = 128
O = 256
NCORES = 8
T = N // NCORES            # 1250 targets per core
SB = (N + 127) // 128      # 79 source blocks
NP = SB * 128              # 10112 padded sources
CH = [(0, 512), (512, 512), (1024, 226)]   # stage-1 t-chunks
FP32 = mybir.dt.float32
BF16 = mybir.dt.bfloat16
FP8 = mybir.dt.float8e4

LAST_RESULTS = None


@functools.lru_cache(maxsize=1)
def build_nc():
    nc = bacc.Bacc(None, target_bir_lowering=False)

    a8_d = nc.dram_tensor("a8", [128, SB, T], FP8, kind="ExternalInput")
    xin_d = nc.dram_tensor("xin", [128, SB, F], BF16, kind="ExternalInput")
    dvs_d = nc.dram_tensor("dvs", [128, SB], FP32, kind="ExternalInput")
    dvt_d = nc.dram_tensor("dvt", [128, T], FP32, kind="ExternalInput")
    w_d = nc.dram_tensor("w", [F, O], FP32, kind="ExternalInput")
    bb_d = nc.dram_tensor("bb", [128, O], FP32, kind="ExternalInput")
    y_d = nc.dram_tensor("y", [T, O], FP32, kind="ExternalOutput")

    with tile.TileContext(nc) as tc:
        with (
            tc.tile_pool(name="res", bufs=1) as res,
            tc.tile_pool(name="ps1", bufs=1, space=bass.MemorySpace.PSUM) as ps1p,
            tc.tile_pool(name="ps2", bufs=2, space=bass.MemorySpace.PSUM) as ps2p,
            tc.tile_pool(name="outp", bufs=3) as outp,
        ):
            xt = res.tile([128, SB, F], BF16, tag="xt")
            xs = res.tile([128, SB, F], BF16, tag="xs")
            at = res.tile([128, SB, T], FP8, tag="at")
            ds = res.tile([128, SB], FP32, tag="ds")
            dt_ = res.tile([128, T], FP32, tag="dt")
            wt = res.tile([F, O], FP32, tag="wt")
            bbt = res.tile([128, O], FP32, tag="bbt")
            aggs = res.tile([128, T], FP32, tag="aggs")

            nc.sync.dma_start(ds[:, :], dvs_d[:, :])
            nc.sync.dma_start(wt[:, :], w_d[:, :])
            nc.sync.dma_start(bbt[:, :], bb_d[:, :])

            # interleave x chunks with A slab chunks so PE can start after
            # the first group; xs scaling follows each x chunk. The first
            # groups are narrow so the first matmuls unblock early.
            AG = 8
            for g in range(0, SB, AG):
                ge = min(SB, g + AG)
                nc.sync.dma_start(xt[:, g:ge, :], xin_d[:, g:ge, :])
                nc.sync.dma_start(at[:, g:ge, :], a8_d[:, g:ge, :])
                for sb in range(g, ge):
                    nc.vector.tensor_scalar_mul(
                        xs[:, sb, :], xt[:, sb, :], ds[:, sb : sb + 1]
                    )
            # dvt is only needed by the stage-1 epilogue — load it last
            nc.sync.dma_start(dt_[:, :], dvt_d[:, :])

            ps = [
                ps1p.tile([128, tn], FP32, tag=f"ps{i}", name=f"ps{i}")
                for i, (t0, tn) in enumerate(CH)
            ]
            for sb in range(SB):
                for i, (t0, tn) in enumerate(CH):
                    nc.tensor.matmul(
                        ps[i][:, :],
                        xs[:, sb, :],
                        at[:, sb, t0 : t0 + tn],
                        start=(sb == 0),
                        stop=(sb == SB - 1),
                    )

            # aggS = agg_T * dinv_t  (dinv of the target, broadcast over partitions)
            for i, (t0, tn) in enumerate(CH):
                nc.vector.tensor_mul(
                    aggs[:, t0 : t0 + tn], ps[i][:, :], dt_[:, t0 : t0 + tn]
                )

            # stage 2: y[t, :] = aggS[:, tblock]^T @ W + b
            for tb in range((T + 127) // 128):
                c0 = tb * 128
                cn = min(128, T - c0)
                p2 = ps2p.tile([cn, O], FP32, tag="p2")
                nc.tensor.matmul(
                    p2[:, :], aggs[:, c0 : c0 + cn], wt[:, :], start=True, stop=True
                )
                ot = outp.tile([cn, O], FP32, tag="ot")
                nc.vector.tensor_add(ot[:, :], p2[:, :], bbt[:cn, :])
                nc.sync.dma_start(y_d[c0 : c0 + cn, :], ot[:, :])

    nc.compile()
    return nc


def _prep_inputs(x, edge_index, W, b):
    x = np.asarray(x, dtype=np.float32)
    edge_index = np.asarray(edge_index)
    W = np.asarray(W, dtype=np.float32)
    b = np.asarray(b, dtype=np.float32)

    row = edge_index[0].astype(np.int64)
    col = edge_index[1].astype(np.int64)

    deg = (np.bincount(col, minlength=N) + 1).astype(np.float32)
    dinv = (1.0 / np.sqrt(deg)).astype(np.float32)

    # dense multiplicity matrix (uint8 counts -> fp8e4m3 bytes via LUT)
    key = row * N + col
    uk, cnt = np.unique(key, return_counts=True)
    A = np.zeros(N * N, dtype=np.uint8)
    A[uk] = cnt.astype(np.uint8)
    A = A.reshape(N, N)
    idx = np.arange(N)
    A[idx, idx] += 1  # self loops
    lut = np.arange(256, dtype=np.float32).astype(ml_dtypes.float8_e4m3).view(np.uint8)
    A8 = lut[A]  # fp8 bytes, shape [N src, N tgt]

    x_pad = np.zeros((NP, F), dtype=np.float32)
    x_pad[:N] = x
    xin = np.ascontiguousarray(
        x_pad.astype(ml_dtypes.bfloat16).reshape(SB, 128, F).transpose(1, 0, 2)
    )
    dinv_pad = np.zeros((NP,), dtype=np.float32)
    dinv_pad[:N] = dinv
    dvs = np.ascontiguousarray(dinv_pad.reshape(SB, 128).T)

    w32 = W.astype(np.float32)
    bb = np.ascontiguousarray(np.broadcast_to(b.astype(np.float32), (128, O)))

    in_maps = []
    for k in range(NCORES):
        sl = A8[:, k * T : (k + 1) * T]  # [N, T]
        slp = np.zeros((NP, T), dtype=np.uint8)
        slp[:N] = sl
        a8 = np.ascontiguousarray(slp.reshape(SB, 128, T).transpose(1, 0, 2)).view(
            ml_dtypes.float8_e4m3
        )
        dvt = np.ascontiguousarray(
            np.broadcast_to(dinv[k * T : (k + 1) * T], (128, T))
        ).astype(np.float32)
        in_maps.append(
            {"a8": a8, "xin": xin, "dvs": dvs, "dvt": dvt, "w": w32, "bb": bb}
        )
    return in_maps


def kernel(x, edge_index, W, b):
    global LAST_RESULTS
    nc = build_nc()
    in_maps = _prep_inputs(x, edge_index, W, b)
    res = run_bass_kernel_spmd(
        nc,
        in_maps,
        core_ids=list(range(NCORES)),
        trace=bool(int(os.environ.get("KERNEL_TRACE", "0"))),
    )
    LAST_RESULTS = res
    out = np.concatenate([r["y"] for r in res.results], axis=0)
    return out.astype(np.float32)
